# revision 52
# baseline (speedup 1.0000x reference)
"""Trainium2 Bass kernel v4 for nn_Attention_65798898975102.

Structure (per core = one batch x one head-group-of-8):
  x-stats (bf16) -> v-proj (fp8 DoubleRow) -> per-head-pair stream:
  [qk-proj (fp8 DR, bias folded into eviction) -> pair rms stats ->
   norm+rope -> attention (bf16 logits; exp split between ScalarE (real
   exp -> f8) and VectorE (Schraudolph bit-trick -> int8 bitcast f8);
   fp8 DR PV) -> Z-normalize direct from PSUM]
  with out-proj interleaved into the last pair. Host sums the two
  head-group partials per batch.

v4 changes vs v3:
  - 1/ln2 folded into the q-side rope tables so the logit matmul
    produces sp = l/ln2; ACT exp uses scale=ln2/8, and a fraction of
    softmax units run on the Vector engine as bits = max(sp+30, 0)
    cast to int8 and bit-viewed as fp8e4m3 (exponential bit trick);
    the constant offset cancels in the Z division.
  - QK bias rank-1 matmuls removed: bias = bqk[col]*sm9row[pos] is
    applied during PSUM eviction via scalar_tensor_tensor with the
    per-partition bqk column and a gpsimd-broadcast sm9 tile.
  - Z-normalization reads o/Z directly from PSUM (recip_approx from
    the PSUM Z row, gpsimd broadcast, single tensor_mul) instead of
    copying to SBUF first.
  - x-stats partial evictions moved to the (idle) Scalar engine.
"""
import sys

sys.path.insert(0, "/opt/trn_rl_repo")

import numpy as np
import ml_dtypes
from contextlib import ExitStack

B, L, H, C, D = 4, 2048, 1024, 1024, 64
NH = 16
EPS = 1e-6
P = 128
NCORES = 8
HG = 2
HPG = NH // HG        # 8
GD = HPG * D          # 512
KC = H // P           # 8
LQ = 4
LK = L // P           # 16
VA_STRIDE = 66
VA_W = 2 * HPG * VA_STRIDE   # 1056
WS = 64.0                    # fp8 weight pre-scale
LN2 = float(np.log(2.0))
EXP_SCALE = LN2 / 8.0        # ACT: v = exp(sp*ln2/8 + bias)
SCH_B = 30.0                 # Schraudolph bits offset; C = 2^((B-56)/8)
EXP_BIAS = (SCH_B - 56.0) / 8.0 * LN2   # ln C
# lk units whose exp runs on the Vector engine, per pair index
OFF_LKS_BY_P4 = [frozenset((3, 7, 11, 15)),
                 frozenset((3, 7, 11, 15)),
                 frozenset((3, 7, 11, 15)),
                 frozenset((3, 7, 11, 15))]

_compiled = [None]
DEBUG = False


def _build():
    import concourse.mybir as mybir
    import concourse.bacc as bacc
    import concourse.tile as tile

    f32 = mybir.dt.float32
    bf16 = mybir.dt.bfloat16
    f8 = mybir.dt.float8e4
    i8 = mybir.dt.int8
    AF = mybir.ActivationFunctionType
    OP = mybir.AluOpType
    PM = mybir.MatmulPerfMode

    nc = bacc.Bacc("TRN2", target_bir_lowering=False, debug=False,
                   num_devices=NCORES)

    xb8 = nc.dram_tensor("xb8", [H, L], f8, kind="ExternalInput").ap()
    wqk8 = nc.dram_tensor("wqk8", [H, 1024], f8, kind="ExternalInput").ap()
    wv8 = nc.dram_tensor("wv8", [H, GD], f8, kind="ExternalInput").ap()
    wout = nc.dram_tensor("wout", [GD, H], bf16, kind="ExternalInput").ap()
    bqk = nc.dram_tensor("bqk", [P, 8], bf16, kind="ExternalInput").ap()
    vbb = nc.dram_tensor("vbb", [P, GD], bf16, kind="ExternalInput").ap()
    ropes = nc.dram_tensor("ropes", [4, P, L], bf16, kind="ExternalInput").ap()
    prot = nc.dram_tensor("prot", [P, P], bf16, kind="ExternalInput").ap()
    selp = nc.dram_tensor("selp", [P, 2 * 4], bf16, kind="ExternalInput").ap()
    selb4 = nc.dram_tensor("selb4", [4, 2 * P], bf16,
                           kind="ExternalInput").ap()
    sel8 = nc.dram_tensor("sel8", [P, 8 * 8], bf16, kind="ExternalInput").ap()
    vaones = nc.dram_tensor("vaones", [P, VA_W], f8, kind="ExternalInput").ap()
    ones8x = nc.dram_tensor("ones8x", [8, 1], bf16, kind="ExternalInput").ap()
    out = nc.dram_tensor("out", [L, H], f32, kind="ExternalOutput").ap()

    with tile.TileContext(nc) as tc, ExitStack() as octx:
        consts = octx.enter_context(
            tc.tile_pool(name="consts", bufs=1, side="left"))
        # tiny consts first (selectors gate the first matmuls)
        selpt = consts.tile([P, 2 * 4], bf16, name="selp")
        nc.sync.dma_start(selpt[:], selp[:])
        selb4t = consts.tile([4, 2 * P], bf16, name="selb4")
        nc.sync.dma_start(selb4t[:], selb4[:])
        sel8t = consts.tile([P, 8 * 8], bf16, name="sel8")
        nc.sync.dma_start(sel8t[:], sel8[:])
        ones8c = consts.tile([8, 1], bf16, name="ones8c")
        nc.sync.dma_start(ones8c[:], ones8x[:])
        prott = consts.tile([P, P], bf16, name="prot")
        nc.sync.dma_start(prott[:], prot[:])
        bqkt = consts.tile([P, 8], bf16, name="bqk")
        nc.sync.dma_start(bqkt[:], bqk[:])
        vbbt = consts.tile([P, GD], bf16, name="vbb")
        nc.sync.dma_start(vbbt[:], vbb[:])
        eps_t = consts.tile([P, 1], f32, name="eps")
        nc.vector.memset(eps_t[:], EPS)
        expb = consts.tile([P, 1], f32, name="expb")
        nc.vector.memset(expb[:], EXP_BIAS)
        # fp8 pair tiles for projections
        xb8t = []
        for j in range(4):
            t = consts.tile([P, 2 * L], f8, name=f"xb8{j}")
            v = t[:].rearrange("p (two l) -> p two l", two=2)
            for sss in range(2):
                r0 = j * 256 + sss * P
                nc.sync.dma_start(v[:, sss, :], xb8[r0:r0 + P, :])
            xb8t.append(v)
        wv8t = []
        for j in range(4):
            t = consts.tile([P, 2 * GD], f8, name=f"wv8{j}")
            v = t[:].rearrange("p (two l) -> p two l", two=2)
            for sss in range(2):
                r0 = j * 256 + sss * P
                nc.sync.dma_start(v[:, sss, :], wv8[r0:r0 + P, :])
            wv8t.append(v)
        wqk8t = []
        for j in range(4):
            t = consts.tile([P, 2 * 1024], f8, name=f"wqk8{j}")
            v = t[:].rearrange("p (two l) -> p two l", two=2)
            for sss in range(2):
                r0 = j * 256 + sss * P
                nc.sync.dma_start(v[:, sss, :], wqk8[r0:r0 + P, :])
            wqk8t.append(v)
        ropet = []
        for i in range(4):
            t = consts.tile([P, L], bf16, name=f"rope{i}")
            nc.sync.dma_start(t[:], ropes[i, :, :])
            ropet.append(t)
        woutt = []
        for j in range(4):
            t = consts.tile([P, 1024], bf16, name=f"wout{j}")
            nc.sync.dma_start(t[:], wout[j * P:(j + 1) * P, :])
            woutt.append(t)

        mid = octx.enter_context(tc.tile_pool(name="mid", bufs=1, side="left"))
        sm9row = mid.tile([1, L], bf16, name="sm9row")
        sm9bc = mid.tile([P, L], bf16, name="sm9bc")
        srw4 = mid.tile([4, L], f32, name="srw4")
        rr4 = mid.tile([4, L], f32, name="rr4")
        rr4b_p = [mid.tile([4, L], bf16, name=f"rr4b{i}") for i in range(4)]
        smc = mid.tile([P, LK], f32, name="smc")
        rc64 = mid.tile([P, LK], f32, name="rc64")   # (1/smc)/WS
        qkt = [mid.tile([P, L], bf16, name=f"qkt{t}") for t in range(8)]
        va = [mid.tile([P, VA_W], f8, name=f"va{pr}") for pr in range(8)]
        onT = [mid.tile([P, L], bf16, name=f"onT{p}") for p in range(4)]
        for pr in range(8):
            nc.sync.dma_start(va[pr][:], vaones[:])

        # ---------------- phase 1: x stats ----------------
        with tc.tile_pool(name="p1sq", bufs=2, side="right") as p1sq, \
             tc.tile_pool(name="p1row", bufs=1, side="right") as p1row, \
             tc.tile_pool(name="p1ps", bufs=1, space="PSUM",
                          side="right") as p1ps, \
             tc.tile_pool(name="p1ps2", bufs=2, space="PSUM",
                          side="right") as p1ps2:
            partS = p1row.tile([8, L], bf16, name="partS")
            rc16 = p1row.tile([P, LK], f32, name="rc16")
            sel8v = sel8t[:].rearrange("p (j c) -> p j c", j=8)
            # batched squares: one [P, L] mul per 128-row block of x, the
            # four per-chunk accumulators live in four PSUM banks
            ppsl = [p1ps.tile([8, 512], f32, tag=f"pps{c}", name=f"pps{c}") for c in range(LQ)]
            for j in range(KC):
                for hf in range(2):
                    hsl = slice(hf * 1024, (hf + 1) * 1024)
                    xsq = p1sq.tile([P, 1024], bf16, tag=f"xsq{hf}",
                                    name=f"xsq{hf}")
                    xsrc = xb8t[j // 2][:, j % 2, hsl]
                    nc.scalar.activation(xsq[:], xsrc, AF.Square)
                    for c2 in range(2):
                        c = hf * 2 + c2
                        nc.tensor.matmul(
                            ppsl[c][:], lhsT=sel8v[:, j, :],
                            rhs=xsq[:, c2 * 512:(c2 + 1) * 512],
                            start=(j == 0), stop=(j == KC - 1))
            for c in range(LQ):
                cs = slice(c * 512, (c + 1) * 512)
                nc.scalar.copy(partS[:, cs], ppsl[c][:])
            for c in range(LQ):
                cs = slice(c * 512, (c + 1) * 512)
                msps = p1ps2.tile([1, 512], f32, tag="msps")
                nc.tensor.matmul(msps[:], lhsT=ones8c[:],
                                 rhs=partS[:, cs], start=True, stop=True)
                nc.scalar.activation(sm9row[:, cs], msps[:], AF.Sqrt,
                                     bias=eps_t[0:1, :], scale=1.0 / H)
            nc.gpsimd.partition_broadcast(sm9bc[:], sm9row[:])
            with tc.tile_pool(name="p1ps3", bufs=1, space="PSUM",
                              side="right") as p1ps3:
                trps = p1ps3.tile([P, LK], f32, name="trps")
                for lk in range(LK):
                    nc.tensor.matmul(
                        trps[:, lk:lk + 1],
                        lhsT=partS[:, lk * P:(lk + 1) * P],
                        rhs=ones8c[:], start=True, stop=True)
                nc.scalar.activation(smc[:], trps[:], AF.Sqrt,
                                     bias=eps_t[:], scale=1.0 / H)
                nc.vector.reciprocal_approx_fast(out=rc16[:], in_=smc[:])
                nc.vector.tensor_scalar_mul(out=rc64[:], in0=rc16[:],
                                            scalar1=1.0 / WS)

        # -------- per-head-pair stream: proj, stats, rope, attention ------
        selpv = selpt[:].rearrange("p (i j) -> p i j", i=2)
        selbv = selb4t[:].rearrange("p (i j) -> p i j", i=2)
        with tc.tile_pool(name="nsq", bufs=2, side="right") as nsq, \
             tc.tile_pool(name="rsc", bufs=2, side="right") as rsc, \
             tc.tile_pool(name="epool", bufs=6, side="right") as epool, \
             tc.tile_pool(name="zsb", bufs=2, side="right") as zsb, \
             tc.tile_pool(name="oevict", bufs=2, side="right") as oevict:

            def qk_proj(cc, qkpsp):
                ccs = slice(cc * P, (cc + 1) * P)
                for lq in range(LQ):
                    lqs = slice(lq * 512, (lq + 1) * 512)
                    ps = qkpsp.tile([P, 512], f32, tag="qk", bufs=3)
                    for j in range(4):
                        nc.tensor.matmul(ps[:], lhsT=wqk8t[j][:, :, ccs],
                                         rhs=xb8t[j][:, :, lqs],
                                         perf_mode=PM.DoubleRow,
                                         start=(j == 0), stop=(j == 3))
                    # evict + bias: qkt = sm9bc*bqk[col] + ps
                    nc.vector.scalar_tensor_tensor(
                        out=qkt[cc][:, lqs], in0=sm9bc[:, lqs],
                        scalar=bqkt[:, cc:cc + 1], in1=ps[:],
                        op0=OP.mult, op1=OP.add)

            def pair_stats(p4, stpsp):
                stl = [stpsp.tile([P, 512], f32, tag=f"st{c}", bufs=1, name=f"st{c}")
                       for c in range(LQ)]
                for i, t in enumerate((p4, 4 + p4)):
                    qsq = nsq.tile([P, L], bf16, tag="qsq")
                    nc.scalar.activation(qsq[:], qkt[t][:], AF.Square)
                    for c in range(LQ):
                        cs = slice(c * 512, (c + 1) * 512)
                        nc.tensor.matmul(stl[c][0:4, :], lhsT=selpv[:, i, :],
                                         rhs=qsq[:, cs],
                                         start=(i == 0), stop=(i == 1))
                for c in range(LQ):
                    cs = slice(c * 512, (c + 1) * 512)
                    nc.scalar.activation(srw4[:, cs], stl[c][0:4, :], AF.Sqrt,
                                         bias=eps_t[0:4, :], scale=1.0 / D)
                nc.vector.reciprocal_approx_fast(out=rr4[:], in_=srw4[:])
                nc.vector.tensor_copy(rr4b_p[p4][:], rr4[:])

            def scale_chunk(t, i, rr4b, c, scrp):
                cs = slice(c * 512, (c + 1) * 512)
                rbc = scrp.tile([P, 512], f32, tag="sp", bufs=3, name="rbc")
                nc.tensor.matmul(rbc[:], lhsT=selbv[:, i, :],
                                 rhs=rr4b[:, cs], start=True, stop=True)
                nc.vector.tensor_mul(qkt[t][:, cs], qkt[t][:, cs], rbc[:])

            def rope_chunk(t, c, scrp):
                cosT = ropet[0] if t < 4 else ropet[2]
                sinT = ropet[1] if t < 4 else ropet[3]
                s2 = slice(c * 512, (c + 1) * 512)
                t1 = rsc.tile([P, 512], bf16, tag="t1", name="t1")
                nc.vector.tensor_mul(t1[:], qkt[t][:, s2], cosT[:, s2])
                pr = scrp.tile([P, 512], f32, tag="sp", bufs=3, name="prx")
                nc.tensor.matmul(pr[:], lhsT=prott[:], rhs=qkt[t][:, s2],
                                 start=True, stop=True)
                t2 = rsc.tile([P, 512], bf16, tag="t2")
                nc.vector.tensor_mul(t2[:], pr[:], sinT[:, s2])
                nc.vector.tensor_add(qkt[t][:, s2], t1[:], t2[:])

            def prep_chunk(p4n, sl, c, scrp):
                if sl == 0:
                    scale_chunk(4 + p4n, 1, rr4b_p[p4n][:], c, scrp)
                elif sl == 1:
                    rope_chunk(4 + p4n, c, scrp)
                elif sl == 2:
                    scale_chunk(p4n, 0, rr4b_p[p4n][:], c, scrp)
                else:
                    rope_chunk(p4n, c, scrp)

            def prep_slice(p4n, sl, scrp):
                for c in range(LQ):
                    prep_chunk(p4n, sl, c, scrp)

            def emit_vchunk(lkv, pool):
                lks = slice(lkv * P, (lkv + 1) * P)
                ps = pool.tile([P, GD], f32, tag="qk", bufs=3,
                               name=f"vps{lkv}")
                for j in range(4):
                    nc.tensor.matmul(ps[:], lhsT=xb8t[j][:, :, lks],
                                     rhs=wv8t[j][:, :, :],
                                     perf_mode=PM.DoubleRow,
                                     start=(j == 0), stop=(j == 3))
                pr, sl = divmod(lkv, 2)
                dst = va[pr][:].rearrange(
                    "p (two h f) -> p two h f", two=2, h=HPG)
                nc.scalar.mul(out=dst[:, sl, :, 0:D],
                              in_=ps[:].rearrange("p (h f) -> p h f", f=D),
                              mul=rc64[:, lkv:lkv + 1])

            # upfront: all qk projections and all pair stats (keeps every
            # Sqrt ahead of the first Exp -> no ACT table churn); stats of
            # pair i overlap the projections of pair i+1.
            with tc.tile_pool(name="upps", bufs=1, space="PSUM",
                              side="right") as upps:
                for cc, stp in ((4, None), (0, None), (5, 0), (1, 1),
                                (6, None), (2, 2), (7, None), (3, 3)):
                    qk_proj(cc, upps)
                    if stp is not None:
                        pair_stats(stp, upps)
                for lkv in range(LK):
                    emit_vchunk(lkv, upps)
                pair_stats(3, upps)
            with tc.tile_pool(name="sps", bufs=3, space="PSUM",
                              side="right") as sps, \
                 tc.tile_pool(name="ops", bufs=1, space="PSUM",
                              side="right") as opsp:
              for sl in range(4):
                  prep_slice(0, sl, sps)
              PREP_CAD = (1, 4, 6, 9, 12, 14)
              PREP_ORDER = [(0, 0), (1, 0), (2, 0), (3, 0),
                            (0, 1), (1, 1), (2, 1), (3, 1),
                            (0, 2), (1, 2), (2, 2), (3, 2),
                            (0, 3), (1, 3), (2, 3), (3, 3)]
              pending_op = []

              def emit_outproj(nsteps):
                  for _ in range(nsteps):
                      if not pending_op:
                          return
                      pos, hc = pending_op.pop(0)
                      po = sps.tile([P, 512], f32, tag="sp", bufs=3)
                      for kk in range(4):
                          nc.tensor.matmul(
                              po[:], lhsT=onT[kk][:, pos],
                              rhs=woutt[kk][:, hc * 512:(hc + 1) * 512],
                              start=(kk == 0), stop=(kk == 3))
                      oe = oevict.tile([P, 512], f32, tag="oe")
                      nc.vector.tensor_copy(oe[:], po[:])
                      nc.sync.dma_start(
                          out[pos, hc * 512:(hc + 1) * 512], oe[:])

              for p4 in range(4):
                  qt = qkt[p4]
                  kt = qkt[4 + p4]
                  h1, h2 = 2 * p4, 2 * p4 + 1
                  for lq in range(LQ):
                      lqs = slice(lq * 512, (lq + 1) * 512)
                      o12 = opsp.tile([D + 1, 1024], f32, tag="o12")
                      o1 = o12[:, 0:512]
                      o2 = o12[:, 512:1024]
                      et2 = None
                      etv = None
                      etv8 = None

                      def emit_pv(prn, etv):
                          vav = va[prn][:].rearrange(
                              "p (two w) -> p two w", two=2)
                          for idx, (hh, ot) in enumerate(
                                  ((h1, o1), (h2, o2))):
                              hc0 = hh * VA_STRIDE
                              nc.tensor.matmul(
                                  ot[:],
                                  lhsT=vav[:, :, hc0:hc0 + D + 1],
                                  rhs=etv[:, :, idx * 512:(idx + 1) * 512],
                                  perf_mode=PM.DoubleRow,
                                  start=(prn == 0), stop=(prn == LK // 2 - 1))

                      pv_q = []
                      for lk in range(LK):
                          lks = slice(lk * P, (lk + 1) * P)
                          sp = sps.tile([P, 1024], f32, tag="sp")
                          nc.tensor.matmul(sp[:, 0:512], lhsT=kt[0:64, lks],
                                           rhs=qt[0:64, lqs],
                                           tile_position=(0, 0),
                                           start=True, stop=True)
                          nc.tensor.matmul(sp[:, 512:1024], lhsT=kt[64:128, lks],
                                           rhs=qt[64:128, lqs],
                                           tile_position=(64, 0),
                                           start=True, stop=True)
                          if p4 < 3 and lk in PREP_CAD:
                              ci = lq * 6 + PREP_CAD.index(lk)
                              if ci < 16:
                                  psl, pc = PREP_ORDER[ci]
                                  prep_chunk(p4 + 1, psl, pc, sps)
                          prn, sl8 = divmod(lk, 2)
                          if sl8 == 0:
                              et2 = epool.tile([P, 2048], f8, tag="et2")
                              etv = et2[:].rearrange(
                                  "p (two q) -> p two q", two=2)
                              etv8 = et2[:].bitcast(i8).rearrange(
                                  "p (two q) -> p two q", two=2)
                              if pending_op and lk >= 2:
                                  emit_outproj(1)
                          if lk in OFF_LKS_BY_P4[p4]:
                              # Schraudolph: bits = max(sp + B, 0) -> int8
                              # (bit pattern of C*exp(l/8) in fp8e4m3)
                              nc.vector.tensor_scalar(
                                  out=etv8[:, sl8, :], in0=sp[:],
                                  scalar1=SCH_B, scalar2=0.0,
                                  op0=OP.add, op1=OP.max)
                          else:
                              nc.scalar.activation(etv[:, sl8, :], sp[:],
                                                   AF.Exp, scale=EXP_SCALE,
                                                   bias=expb[:])
                          if sl8 == 1:
                              # defer PV by 2 et-pairs so the PE never
                              # head-of-line blocks on a fresh exp
                              pv_q.append((prn, etv))
                              if len(pv_q) >= 3:
                                  emit_pv(*pv_q.pop(0))
                      for item in pv_q:
                          emit_pv(*item)
                      zrow = zsb.tile([1, 1024], f32, tag="zrow", bufs=1)
                      nc.vector.tensor_copy(zrow[:], o12[D:D + 1, :])
                      rz = zsb.tile([1, 1024], f32, tag="rz", bufs=1)
                      nc.vector.reciprocal_approx_fast(out=rz[:], in_=zrow[:])
                      bz = zsb.tile([64, 1024], f32, tag="bz")
                      nc.gpsimd.partition_broadcast(bz[:], rz[:])
                      for hh in (0, 1):
                          hs = slice(hh * 512, (hh + 1) * 512)
                          nc.vector.tensor_mul(
                              onT[p4][hh * 64:(hh + 1) * 64, lqs],
                              o12[0:D, hs], bz[:, hs])
                      if p4 >= 3:
                          for lq16 in range(4):
                              pos = slice(lq * 512 + lq16 * P,
                                          lq * 512 + (lq16 + 1) * P)
                              for hc in range(2):
                                  pending_op.append((pos, hc))
              emit_outproj(len(pending_op) + 1)

    nc.compile()
    return nc


def _host_prep(x, condition, rope, W_ada, b_ada, W_qkv, W_out, q_scale,
               k_scale):
    bf = ml_dtypes.bfloat16
    f8 = ml_dtypes.float8_e4m3
    x = np.asarray(x, np.float32)
    cond = np.asarray(condition, np.float64)[:, 0, :]
    ada = cond @ np.asarray(W_ada, np.float64) + np.asarray(b_ada, np.float64)
    shift = ada[:, :H]
    scale1 = ada[:, H:] + 1.0

    Wq = np.asarray(W_qkv, np.float64)[:, 0:H]
    Wk = np.asarray(W_qkv, np.float64)[:, H:2 * H]
    Wv = np.asarray(W_qkv, np.float64)[:, 2 * H:3 * H]
    Wo = np.asarray(W_out, np.float64)

    cos = np.asarray(rope, np.float64)[0, 0, :, 0, :]
    sin = np.asarray(rope, np.float64)[1, 0, :, 0, :]
    qs = np.asarray(q_scale, np.float64)
    ks = np.asarray(k_scale, np.float64)
    qs_sw = qs.reshape(-1, 2)[:, ::-1].ravel()
    ks_sw = ks.reshape(-1, 2)[:, ::-1].ravel()

    def rope_tiles(s, s_sw):
        cT = (cos * s[None, :]).T
        sT = (sin * s_sw[None, :]).T
        return (np.concatenate([cT, cT], 0), np.concatenate([sT, sT], 0))

    # fold 1/ln2 into the q-side tables: logits come out as l/ln2
    cq2, sq2 = rope_tiles(qs / LN2, qs_sw / LN2)
    ck2, sk2 = rope_tiles(ks, ks_sw)
    ropes_a = np.stack([cq2, sq2, ck2, sk2]).astype(bf)

    prot = np.zeros((P, P), np.float32)
    for i in range(P // 2):
        prot[2 * i + 1, 2 * i] = -1.0
        prot[2 * i, 2 * i + 1] = 1.0

    selp = np.zeros((P, 2, 4), np.float32)
    for i in range(2):
        selp[0:64, i, 2 * i] = 1.0
        selp[64:128, i, 2 * i + 1] = 1.0
    selb4 = np.zeros((4, 2, P), np.float32)
    for i in range(2):
        selb4[2 * i, i, 0:64] = 1.0
        selb4[2 * i + 1, i, 64:128] = 1.0
    sel8 = np.zeros((P, 8, 8), np.float32)
    for j in range(8):
        sel8[:, j, j] = 1.0
    vaones = np.ones((P, VA_W), np.float32).astype(f8)

    in_maps = []
    bv_corrs = []
    for core in range(NCORES):
        b, g = divmod(core, HG)
        gsl = slice(g * GD, (g + 1) * GD)
        sc_b = scale1[b][:, None]
        wq_eff = sc_b * Wq[:, gsl]
        wk_eff = sc_b * Wk[:, gsl]
        wv_eff = sc_b * Wv[:, gsl]
        bq = shift[b] @ Wq[:, gsl]
        bk = shift[b] @ Wk[:, gsl]
        bv = shift[b] @ Wv[:, gsl]
        xT = np.ascontiguousarray(x[b].T)
        bqk_full = np.concatenate([bq, bk]) * WS          # [1024]
        bv_corrs.append(bv @ Wo[gsl, :])                  # [1024]
        in_maps.append({
            "xb8": xT.astype(f8),
            "wqk8": (np.concatenate([wq_eff, wk_eff], 1) * WS).astype(f8),
            "wv8": np.ascontiguousarray(wv_eff * WS).astype(f8),
            "wout": np.ascontiguousarray(Wo[gsl, :]).astype(bf),
            "bqk": np.ascontiguousarray(
                bqk_full.reshape(8, P).T).astype(bf),   # [128, 8] per chunk
            "vbb": np.broadcast_to(bv, (P, GD)).astype(bf),
            "ropes": ropes_a,
            "prot": prot.astype(bf),
            "selp": selp.reshape(P, 8).astype(bf),
            "selb4": selb4.reshape(4, 2 * P).astype(bf),
            "sel8": sel8.reshape(P, 64).astype(bf),
            "vaones": vaones,
            "ones8x": np.ones((8, 1), np.float32).astype(bf),
        })
    return in_maps, bv_corrs


def kernel(x, condition, rope, W_ada, b_ada, W_qkv, W_out, q_scale, k_scale,
           _trace=False, _tmpdir=None):
    from concourse import bass_utils

    if _compiled[0] is None:
        _compiled[0] = _build()
    nc = _compiled[0]

    in_maps, bv_corrs = _host_prep(x, condition, rope, W_ada, b_ada,
                                   W_qkv, W_out, q_scale, k_scale)
    kw = {}
    if _trace:
        kw = {"trace": True, "tmpdir": _tmpdir}
    res = bass_utils.run_bass_kernel_spmd(
        nc, in_maps, core_ids=list(range(NCORES)), **kw)

    full = np.empty((B, L, H), np.float32)
    for b in range(B):
        corr = (bv_corrs[2 * b] + bv_corrs[2 * b + 1]).astype(np.float32)
        full[b] = (res.results[2 * b]["out"] +
                   res.results[2 * b + 1]["out"] + corr[None, :])
    if _trace:
        return full, res
    return full


# revision 53
# speedup vs baseline: 1.0153x; 1.0153x over previous
"""Trainium2 Bass kernel v4 for nn_Attention_65798898975102.

Structure (per core = one batch x one head-group-of-8):
  x-stats (bf16) -> v-proj (fp8 DoubleRow) -> per-head-pair stream:
  [qk-proj (fp8 DR, bias folded into eviction) -> pair rms stats ->
   norm+rope -> attention (bf16 logits; exp split between ScalarE (real
   exp -> f8) and VectorE (Schraudolph bit-trick -> int8 bitcast f8);
   fp8 DR PV) -> Z-normalize direct from PSUM]
  with out-proj interleaved into the last pair. Host sums the two
  head-group partials per batch.

v4 changes vs v3:
  - 1/ln2 folded into the q-side rope tables so the logit matmul
    produces sp = l/ln2; ACT exp uses scale=ln2/8, and a fraction of
    softmax units run on the Vector engine as bits = max(sp+30, 0)
    cast to int8 and bit-viewed as fp8e4m3 (exponential bit trick);
    the constant offset cancels in the Z division.
  - QK bias rank-1 matmuls removed: bias = bqk[col]*sm9row[pos] is
    applied during PSUM eviction via scalar_tensor_tensor with the
    per-partition bqk column and a gpsimd-broadcast sm9 tile.
  - Z-normalization reads o/Z directly from PSUM (recip_approx from
    the PSUM Z row, gpsimd broadcast, single tensor_mul) instead of
    copying to SBUF first.
  - x-stats partial evictions moved to the (idle) Scalar engine.
"""
import sys

sys.path.insert(0, "/opt/trn_rl_repo")

import numpy as np
import ml_dtypes
from contextlib import ExitStack

B, L, H, C, D = 4, 2048, 1024, 1024, 64
NH = 16
EPS = 1e-6
P = 128
NCORES = 8
HG = 2
HPG = NH // HG        # 8
GD = HPG * D          # 512
KC = H // P           # 8
LQ = 4
LK = L // P           # 16
VA_STRIDE = 66
VA_W = 2 * HPG * VA_STRIDE   # 1056
WS = 64.0                    # fp8 weight pre-scale
LN2 = float(np.log(2.0))
EXP_SCALE = LN2 / 8.0        # ACT: v = exp(sp*ln2/8 + bias)
SCH_B = 30.0                 # Schraudolph bits offset; C = 2^((B-56)/8)
EXP_BIAS = (SCH_B - 56.0) / 8.0 * LN2   # ln C
# lk units whose exp runs on the Vector engine, per pair index
OFF_LKS_BY_P4 = [frozenset((3, 7, 11, 15)),
                 frozenset((3, 7, 11, 15)),
                 frozenset((3, 7, 11, 15)),
                 frozenset((3, 7, 11, 15))]

_compiled = [None]
DEBUG = False


def _build():
    import concourse.mybir as mybir
    import concourse.bacc as bacc
    import concourse.tile as tile

    f32 = mybir.dt.float32
    bf16 = mybir.dt.bfloat16
    f8 = mybir.dt.float8e4
    i8 = mybir.dt.int8
    AF = mybir.ActivationFunctionType
    OP = mybir.AluOpType
    PM = mybir.MatmulPerfMode

    nc = bacc.Bacc("TRN2", target_bir_lowering=False, debug=False,
                   num_devices=NCORES)

    xb8 = nc.dram_tensor("xb8", [H, L], f8, kind="ExternalInput").ap()
    wqk8 = nc.dram_tensor("wqk8", [H, 1024], f8, kind="ExternalInput").ap()
    wv8 = nc.dram_tensor("wv8", [H, GD], f8, kind="ExternalInput").ap()
    wout = nc.dram_tensor("wout", [GD, H], bf16, kind="ExternalInput").ap()
    bqk = nc.dram_tensor("bqk", [P, 8], bf16, kind="ExternalInput").ap()
    vbb = nc.dram_tensor("vbb", [P, GD], bf16, kind="ExternalInput").ap()
    ropes = nc.dram_tensor("ropes", [4, P, L], bf16, kind="ExternalInput").ap()
    prot = nc.dram_tensor("prot", [P, P], bf16, kind="ExternalInput").ap()
    selp = nc.dram_tensor("selp", [P, 2 * 4], bf16, kind="ExternalInput").ap()
    selb4 = nc.dram_tensor("selb4", [4, 2 * P], bf16,
                           kind="ExternalInput").ap()
    sel8 = nc.dram_tensor("sel8", [P, 8 * 8], bf16, kind="ExternalInput").ap()
    vaones = nc.dram_tensor("vaones", [P, VA_W], f8, kind="ExternalInput").ap()
    ones8x = nc.dram_tensor("ones8x", [8, 1], bf16, kind="ExternalInput").ap()
    out = nc.dram_tensor("out", [L, H], f32, kind="ExternalOutput").ap()

    with tile.TileContext(nc) as tc, ExitStack() as octx:
        consts = octx.enter_context(
            tc.tile_pool(name="consts", bufs=1, side="left"))
        # tiny consts first (selectors gate the first matmuls)
        selpt = consts.tile([P, 2 * 4], bf16, name="selp")
        nc.sync.dma_start(selpt[:], selp[:])
        selb4t = consts.tile([4, 2 * P], bf16, name="selb4")
        nc.sync.dma_start(selb4t[:], selb4[:])
        sel8t = consts.tile([P, 8 * 8], bf16, name="sel8")
        nc.sync.dma_start(sel8t[:], sel8[:])
        ones8c = consts.tile([8, 1], bf16, name="ones8c")
        nc.sync.dma_start(ones8c[:], ones8x[:])
        prott = consts.tile([P, P], bf16, name="prot")
        nc.sync.dma_start(prott[:], prot[:])
        bqkt = consts.tile([P, 8], bf16, name="bqk")
        nc.sync.dma_start(bqkt[:], bqk[:])
        vbbt = consts.tile([P, GD], bf16, name="vbb")
        nc.sync.dma_start(vbbt[:], vbb[:])
        eps_t = consts.tile([P, 1], f32, name="eps")
        nc.vector.memset(eps_t[:], EPS)
        expb = consts.tile([P, 1], f32, name="expb")
        nc.vector.memset(expb[:], EXP_BIAS)
        # fp8 pair tiles for projections
        xb8t = []
        for j in range(4):
            t = consts.tile([P, 2 * L], f8, name=f"xb8{j}")
            v = t[:].rearrange("p (two l) -> p two l", two=2)
            for sss in range(2):
                r0 = j * 256 + sss * P
                nc.sync.dma_start(v[:, sss, :], xb8[r0:r0 + P, :])
            xb8t.append(v)
        wv8t = []
        for j in range(4):
            t = consts.tile([P, 2 * GD], f8, name=f"wv8{j}")
            v = t[:].rearrange("p (two l) -> p two l", two=2)
            for sss in range(2):
                r0 = j * 256 + sss * P
                nc.sync.dma_start(v[:, sss, :], wv8[r0:r0 + P, :])
            wv8t.append(v)
        wqk8t = []
        for j in range(4):
            t = consts.tile([P, 2 * 1024], f8, name=f"wqk8{j}")
            v = t[:].rearrange("p (two l) -> p two l", two=2)
            for sss in range(2):
                r0 = j * 256 + sss * P
                nc.sync.dma_start(v[:, sss, :], wqk8[r0:r0 + P, :])
            wqk8t.append(v)
        ropet = []
        for i in range(4):
            t = consts.tile([P, L], bf16, name=f"rope{i}")
            nc.sync.dma_start(t[:], ropes[i, :, :])
            ropet.append(t)
        woutt = []
        for j in range(4):
            t = consts.tile([P, 1024], bf16, name=f"wout{j}")
            nc.sync.dma_start(t[:], wout[j * P:(j + 1) * P, :])
            woutt.append(t)

        mid = octx.enter_context(tc.tile_pool(name="mid", bufs=1, side="left"))
        sm9row = mid.tile([1, L], bf16, name="sm9row")
        sm9bc = mid.tile([P, L], bf16, name="sm9bc")
        srw4 = mid.tile([4, L], f32, name="srw4")
        rr4 = mid.tile([4, L], f32, name="rr4")
        rr4b_p = [mid.tile([4, L], bf16, name=f"rr4b{i}") for i in range(4)]
        smc = mid.tile([P, LK], f32, name="smc")
        rc64 = mid.tile([P, LK], f32, name="rc64")   # (1/smc)/WS
        qkt = [mid.tile([P, L], bf16, name=f"qkt{t}") for t in range(8)]
        va = [mid.tile([P, VA_W], f8, name=f"va{pr}") for pr in range(8)]
        onT = [mid.tile([P, L], bf16, name=f"onT{p}") for p in range(4)]
        for pr in range(8):
            nc.sync.dma_start(va[pr][:], vaones[:])

        # ---------------- phase 1: x stats ----------------
        with tc.tile_pool(name="p1sq", bufs=2, side="right") as p1sq, \
             tc.tile_pool(name="p1row", bufs=1, side="right") as p1row, \
             tc.tile_pool(name="p1ps", bufs=1, space="PSUM",
                          side="right") as p1ps, \
             tc.tile_pool(name="p1ps2", bufs=2, space="PSUM",
                          side="right") as p1ps2:
            partS = p1row.tile([8, L], bf16, name="partS")
            rc16 = p1row.tile([P, LK], f32, name="rc16")
            sel8v = sel8t[:].rearrange("p (j c) -> p j c", j=8)
            # batched squares: one [P, L] mul per 128-row block of x, the
            # four per-chunk accumulators live in four PSUM banks
            ppsl = [p1ps.tile([8, 512], f32, tag=f"pps{c}", name=f"pps{c}") for c in range(LQ)]
            for j in range(KC):
                for hf in range(2):
                    hsl = slice(hf * 1024, (hf + 1) * 1024)
                    xsq = p1sq.tile([P, 1024], bf16, tag=f"xsq{hf}",
                                    name=f"xsq{hf}")
                    xsrc = xb8t[j // 2][:, j % 2, hsl]
                    nc.scalar.activation(xsq[:], xsrc, AF.Square)
                    for c2 in range(2):
                        c = hf * 2 + c2
                        nc.tensor.matmul(
                            ppsl[c][:], lhsT=sel8v[:, j, :],
                            rhs=xsq[:, c2 * 512:(c2 + 1) * 512],
                            start=(j == 0), stop=(j == KC - 1))
            for c in range(LQ):
                cs = slice(c * 512, (c + 1) * 512)
                nc.scalar.copy(partS[:, cs], ppsl[c][:])
            for c in range(LQ):
                cs = slice(c * 512, (c + 1) * 512)
                msps = p1ps2.tile([1, 512], f32, tag="msps")
                nc.tensor.matmul(msps[:], lhsT=ones8c[:],
                                 rhs=partS[:, cs], start=True, stop=True)
                nc.scalar.activation(sm9row[:, cs], msps[:], AF.Sqrt,
                                     bias=eps_t[0:1, :], scale=1.0 / H)
            nc.gpsimd.partition_broadcast(sm9bc[:], sm9row[:])
            with tc.tile_pool(name="p1ps3", bufs=1, space="PSUM",
                              side="right") as p1ps3:
                trps = p1ps3.tile([P, LK], f32, name="trps")
                for lk in range(LK):
                    nc.tensor.matmul(
                        trps[:, lk:lk + 1],
                        lhsT=partS[:, lk * P:(lk + 1) * P],
                        rhs=ones8c[:], start=True, stop=True)
                nc.scalar.activation(smc[:], trps[:], AF.Sqrt,
                                     bias=eps_t[:], scale=1.0 / H)
                nc.vector.reciprocal_approx_fast(out=rc16[:], in_=smc[:])
                nc.vector.tensor_scalar_mul(out=rc64[:], in0=rc16[:],
                                            scalar1=1.0 / WS)

        # -------- per-head-pair stream: proj, stats, rope, attention ------
        selpv = selpt[:].rearrange("p (i j) -> p i j", i=2)
        selbv = selb4t[:].rearrange("p (i j) -> p i j", i=2)
        with tc.tile_pool(name="nsq", bufs=2, side="right") as nsq, \
             tc.tile_pool(name="rsc", bufs=2, side="right") as rsc, \
             tc.tile_pool(name="epool", bufs=6, side="right") as epool, \
             tc.tile_pool(name="zsb", bufs=2, side="right") as zsb, \
             tc.tile_pool(name="oevict", bufs=2, side="right") as oevict:

            def qk_proj(cc, qkpsp):
                ccs = slice(cc * P, (cc + 1) * P)
                for lq in range(LQ):
                    lqs = slice(lq * 512, (lq + 1) * 512)
                    ps = qkpsp.tile([P, 512], f32, tag="qk", bufs=3)
                    for j in range(4):
                        nc.tensor.matmul(ps[:], lhsT=wqk8t[j][:, :, ccs],
                                         rhs=xb8t[j][:, :, lqs],
                                         perf_mode=PM.DoubleRow,
                                         start=(j == 0), stop=(j == 3))
                    # evict + bias: qkt = sm9bc*bqk[col] + ps
                    nc.vector.scalar_tensor_tensor(
                        out=qkt[cc][:, lqs], in0=sm9bc[:, lqs],
                        scalar=bqkt[:, cc:cc + 1], in1=ps[:],
                        op0=OP.mult, op1=OP.add)

            def pair_stats(p4, stpsp):
                stl = [stpsp.tile([P, 512], f32, tag=f"st{c}", bufs=1, name=f"st{c}")
                       for c in range(LQ)]
                for i, t in enumerate((p4, 4 + p4)):
                    qsq = nsq.tile([P, L], bf16, tag="qsq")
                    nc.scalar.activation(qsq[:], qkt[t][:], AF.Square)
                    for c in range(LQ):
                        cs = slice(c * 512, (c + 1) * 512)
                        nc.tensor.matmul(stl[c][0:4, :], lhsT=selpv[:, i, :],
                                         rhs=qsq[:, cs],
                                         start=(i == 0), stop=(i == 1))
                for c in range(LQ):
                    cs = slice(c * 512, (c + 1) * 512)
                    nc.scalar.activation(srw4[:, cs], stl[c][0:4, :], AF.Sqrt,
                                         bias=eps_t[0:4, :], scale=1.0 / D)
                nc.vector.reciprocal_approx_fast(out=rr4[:], in_=srw4[:])
                nc.vector.tensor_copy(rr4b_p[p4][:], rr4[:])

            def scale_chunk(t, i, rr4b, c, scrp):
                cs = slice(c * 512, (c + 1) * 512)
                rbc = scrp.tile([P, 512], f32, tag="sp", bufs=3, name="rbc")
                nc.tensor.matmul(rbc[:], lhsT=selbv[:, i, :],
                                 rhs=rr4b[:, cs], start=True, stop=True)
                nc.vector.tensor_mul(qkt[t][:, cs], qkt[t][:, cs], rbc[:])

            def rope_chunk(t, c, scrp):
                cosT = ropet[0] if t < 4 else ropet[2]
                sinT = ropet[1] if t < 4 else ropet[3]
                s2 = slice(c * 512, (c + 1) * 512)
                t1 = rsc.tile([P, 512], bf16, tag="t1", name="t1")
                nc.vector.tensor_mul(t1[:], qkt[t][:, s2], cosT[:, s2])
                pr = scrp.tile([P, 512], f32, tag="sp", bufs=3, name="prx")
                nc.tensor.matmul(pr[:], lhsT=prott[:], rhs=qkt[t][:, s2],
                                 start=True, stop=True)
                t2 = rsc.tile([P, 512], bf16, tag="t2")
                nc.vector.tensor_mul(t2[:], pr[:], sinT[:, s2])
                nc.vector.tensor_add(qkt[t][:, s2], t1[:], t2[:])

            def prep_chunk(p4n, sl, c, scrp):
                if sl == 0:
                    scale_chunk(4 + p4n, 1, rr4b_p[p4n][:], c, scrp)
                elif sl == 1:
                    rope_chunk(4 + p4n, c, scrp)
                elif sl == 2:
                    scale_chunk(p4n, 0, rr4b_p[p4n][:], c, scrp)
                else:
                    rope_chunk(p4n, c, scrp)

            def prep_slice(p4n, sl, scrp):
                for c in range(LQ):
                    prep_chunk(p4n, sl, c, scrp)

            def emit_vchunk(lkv, pool):
                lks = slice(lkv * P, (lkv + 1) * P)
                ps = pool.tile([P, GD], f32, tag="qk", bufs=3,
                               name=f"vps{lkv}")
                for j in range(4):
                    nc.tensor.matmul(ps[:], lhsT=xb8t[j][:, :, lks],
                                     rhs=wv8t[j][:, :, :],
                                     perf_mode=PM.DoubleRow,
                                     start=(j == 0), stop=(j == 3))
                pr, sl = divmod(lkv, 2)
                dst = va[pr][:].rearrange(
                    "p (two h f) -> p two h f", two=2, h=HPG)
                nc.vector.scalar_tensor_tensor(
                    out=dst[:, sl, :, 0:D],
                    in0=ps[:].rearrange("p (h f) -> p h f", f=D),
                    scalar=rc64[:, lkv:lkv + 1],
                    in1=vbbt[:].rearrange("p (h f) -> p h f", f=D),
                    op0=OP.mult, op1=OP.add)

            # upfront: all qk projections and all pair stats (keeps every
            # Sqrt ahead of the first Exp -> no ACT table churn); stats of
            # pair i overlap the projections of pair i+1.
            with tc.tile_pool(name="upps", bufs=1, space="PSUM",
                              side="right") as upps:
                for cc, stp in ((4, None), (0, None), (5, 0), (1, 1),
                                (6, None), (2, 2), (7, None), (3, 3)):
                    qk_proj(cc, upps)
                    if stp is not None:
                        pair_stats(stp, upps)
                for lkv in range(LK):
                    emit_vchunk(lkv, upps)
                pair_stats(3, upps)
            with tc.tile_pool(name="sps", bufs=3, space="PSUM",
                              side="right") as sps, \
                 tc.tile_pool(name="ops", bufs=1, space="PSUM",
                              side="right") as opsp:
              for sl in range(4):
                  prep_slice(0, sl, sps)
              PREP_CAD = (1, 4, 6, 9, 12, 14)
              PREP_ORDER = [(0, 0), (1, 0), (2, 0), (3, 0),
                            (0, 1), (1, 1), (2, 1), (3, 1),
                            (0, 2), (1, 2), (2, 2), (3, 2),
                            (0, 3), (1, 3), (2, 3), (3, 3)]
              pending_op = []

              def emit_outproj(nsteps):
                  for _ in range(nsteps):
                      if not pending_op:
                          return
                      pos, hc = pending_op.pop(0)
                      po = sps.tile([P, 512], f32, tag="sp", bufs=3)
                      for kk in range(4):
                          nc.tensor.matmul(
                              po[:], lhsT=onT[kk][:, pos],
                              rhs=woutt[kk][:, hc * 512:(hc + 1) * 512],
                              start=(kk == 0), stop=(kk == 3))
                      oe = oevict.tile([P, 512], f32, tag="oe")
                      nc.vector.tensor_copy(oe[:], po[:])
                      nc.sync.dma_start(
                          out[pos, hc * 512:(hc + 1) * 512], oe[:])

              for p4 in range(4):
                  qt = qkt[p4]
                  kt = qkt[4 + p4]
                  h1, h2 = 2 * p4, 2 * p4 + 1
                  for lq in range(LQ):
                      lqs = slice(lq * 512, (lq + 1) * 512)
                      o12 = opsp.tile([D + 1, 1024], f32, tag="o12")
                      o1 = o12[:, 0:512]
                      o2 = o12[:, 512:1024]
                      et2 = None
                      etv = None
                      etv8 = None

                      def emit_pv(prn, etv):
                          vav = va[prn][:].rearrange(
                              "p (two w) -> p two w", two=2)
                          for idx, (hh, ot) in enumerate(
                                  ((h1, o1), (h2, o2))):
                              hc0 = hh * VA_STRIDE
                              nc.tensor.matmul(
                                  ot[:],
                                  lhsT=vav[:, :, hc0:hc0 + D + 1],
                                  rhs=etv[:, :, idx * 512:(idx + 1) * 512],
                                  perf_mode=PM.DoubleRow,
                                  start=(prn == 0), stop=(prn == LK // 2 - 1))

                      pv_q = []
                      for lk in range(LK):
                          lks = slice(lk * P, (lk + 1) * P)
                          sp = sps.tile([P, 1024], f32, tag="sp")
                          nc.tensor.matmul(sp[:, 0:512], lhsT=kt[0:64, lks],
                                           rhs=qt[0:64, lqs],
                                           tile_position=(0, 0),
                                           start=True, stop=True)
                          nc.tensor.matmul(sp[:, 512:1024], lhsT=kt[64:128, lks],
                                           rhs=qt[64:128, lqs],
                                           tile_position=(64, 0),
                                           start=True, stop=True)
                          if p4 < 3 and lk in PREP_CAD:
                              ci = lq * 6 + PREP_CAD.index(lk)
                              if ci < 16:
                                  psl, pc = PREP_ORDER[ci]
                                  prep_chunk(p4 + 1, psl, pc, sps)
                          prn, sl8 = divmod(lk, 2)
                          if sl8 == 0:
                              et2 = epool.tile([P, 2048], f8, tag="et2")
                              etv = et2[:].rearrange(
                                  "p (two q) -> p two q", two=2)
                              etv8 = et2[:].bitcast(i8).rearrange(
                                  "p (two q) -> p two q", two=2)
                              if pending_op and lk >= 2:
                                  emit_outproj(1)
                          if lk in OFF_LKS_BY_P4[p4]:
                              # Schraudolph: bits = max(sp + B, 0) -> int8
                              # (bit pattern of C*exp(l/8) in fp8e4m3)
                              nc.vector.tensor_scalar(
                                  out=etv8[:, sl8, :], in0=sp[:],
                                  scalar1=SCH_B, scalar2=0.0,
                                  op0=OP.add, op1=OP.max)
                          else:
                              nc.scalar.activation(etv[:, sl8, :], sp[:],
                                                   AF.Exp, scale=EXP_SCALE,
                                                   bias=expb[:])
                          if sl8 == 1:
                              # defer PV by 2 et-pairs so the PE never
                              # head-of-line blocks on a fresh exp
                              pv_q.append((prn, etv))
                              if len(pv_q) >= 3:
                                  emit_pv(*pv_q.pop(0))
                      for item in pv_q:
                          emit_pv(*item)
                      zrow = zsb.tile([1, 1024], f32, tag="zrow", bufs=1)
                      nc.vector.tensor_copy(zrow[:], o12[D:D + 1, :])
                      rz = zsb.tile([1, 1024], f32, tag="rz", bufs=1)
                      nc.vector.reciprocal_approx_fast(out=rz[:], in_=zrow[:])
                      bz = zsb.tile([64, 1024], f32, tag="bz")
                      nc.gpsimd.partition_broadcast(bz[:], rz[:])
                      for hh in (0, 1):
                          hs = slice(hh * 512, (hh + 1) * 512)
                          nc.vector.tensor_mul(
                              onT[p4][hh * 64:(hh + 1) * 64, lqs],
                              o12[0:D, hs], bz[:, hs])
                      if p4 >= 3:
                          for lq16 in range(4):
                              pos = slice(lq * 512 + lq16 * P,
                                          lq * 512 + (lq16 + 1) * P)
                              for hc in range(2):
                                  pending_op.append((pos, hc))
              emit_outproj(len(pending_op) + 1)

    nc.compile()
    return nc


def _host_prep(x, condition, rope, W_ada, b_ada, W_qkv, W_out, q_scale,
               k_scale):
    bf = ml_dtypes.bfloat16
    f8 = ml_dtypes.float8_e4m3
    x = np.asarray(x, np.float32)
    cond = np.asarray(condition, np.float64)[:, 0, :]
    ada = cond @ np.asarray(W_ada, np.float64) + np.asarray(b_ada, np.float64)
    shift = ada[:, :H]
    scale1 = ada[:, H:] + 1.0

    Wq = np.asarray(W_qkv, np.float64)[:, 0:H]
    Wk = np.asarray(W_qkv, np.float64)[:, H:2 * H]
    Wv = np.asarray(W_qkv, np.float64)[:, 2 * H:3 * H]
    Wo = np.asarray(W_out, np.float64)

    cos = np.asarray(rope, np.float64)[0, 0, :, 0, :]
    sin = np.asarray(rope, np.float64)[1, 0, :, 0, :]
    qs = np.asarray(q_scale, np.float64)
    ks = np.asarray(k_scale, np.float64)
    qs_sw = qs.reshape(-1, 2)[:, ::-1].ravel()
    ks_sw = ks.reshape(-1, 2)[:, ::-1].ravel()

    def rope_tiles(s, s_sw):
        cT = (cos * s[None, :]).T
        sT = (sin * s_sw[None, :]).T
        return (np.concatenate([cT, cT], 0), np.concatenate([sT, sT], 0))

    # fold 1/ln2 into the q-side tables: logits come out as l/ln2
    cq2, sq2 = rope_tiles(qs / LN2, qs_sw / LN2)
    ck2, sk2 = rope_tiles(ks, ks_sw)
    ropes_a = np.stack([cq2, sq2, ck2, sk2]).astype(bf)

    prot = np.zeros((P, P), np.float32)
    for i in range(P // 2):
        prot[2 * i + 1, 2 * i] = -1.0
        prot[2 * i, 2 * i + 1] = 1.0

    selp = np.zeros((P, 2, 4), np.float32)
    for i in range(2):
        selp[0:64, i, 2 * i] = 1.0
        selp[64:128, i, 2 * i + 1] = 1.0
    selb4 = np.zeros((4, 2, P), np.float32)
    for i in range(2):
        selb4[2 * i, i, 0:64] = 1.0
        selb4[2 * i + 1, i, 64:128] = 1.0
    sel8 = np.zeros((P, 8, 8), np.float32)
    for j in range(8):
        sel8[:, j, j] = 1.0
    vaones = np.ones((P, VA_W), np.float32).astype(f8)

    in_maps = []
    for core in range(NCORES):
        b, g = divmod(core, HG)
        gsl = slice(g * GD, (g + 1) * GD)
        sc_b = scale1[b][:, None]
        wq_eff = sc_b * Wq[:, gsl]
        wk_eff = sc_b * Wk[:, gsl]
        wv_eff = sc_b * Wv[:, gsl]
        bq = shift[b] @ Wq[:, gsl]
        bk = shift[b] @ Wk[:, gsl]
        bv = shift[b] @ Wv[:, gsl]
        xT = np.ascontiguousarray(x[b].T)
        bqk_full = np.concatenate([bq, bk]) * WS          # [1024]
        in_maps.append({
            "xb8": xT.astype(f8),
            "wqk8": (np.concatenate([wq_eff, wk_eff], 1) * WS).astype(f8),
            "wv8": np.ascontiguousarray(wv_eff * WS).astype(f8),
            "wout": np.ascontiguousarray(Wo[gsl, :]).astype(bf),
            "bqk": np.ascontiguousarray(
                bqk_full.reshape(8, P).T).astype(bf),   # [128, 8] per chunk
            "vbb": np.broadcast_to(bv, (P, GD)).astype(bf),
            "ropes": ropes_a,
            "prot": prot.astype(bf),
            "selp": selp.reshape(P, 8).astype(bf),
            "selb4": selb4.reshape(4, 2 * P).astype(bf),
            "sel8": sel8.reshape(P, 64).astype(bf),
            "vaones": vaones,
            "ones8x": np.ones((8, 1), np.float32).astype(bf),
        })
    return in_maps


def kernel(x, condition, rope, W_ada, b_ada, W_qkv, W_out, q_scale, k_scale,
           _trace=False, _tmpdir=None):
    from concourse import bass_utils

    if _compiled[0] is None:
        _compiled[0] = _build()
    nc = _compiled[0]

    in_maps = _host_prep(x, condition, rope, W_ada, b_ada, W_qkv, W_out,
                         q_scale, k_scale)
    kw = {}
    if _trace:
        kw = {"trace": True, "tmpdir": _tmpdir}
    res = bass_utils.run_bass_kernel_spmd(
        nc, in_maps, core_ids=list(range(NCORES)), **kw)

    full = np.empty((B, L, H), np.float32)
    for b in range(B):
        full[b] = res.results[2 * b]["out"] + res.results[2 * b + 1]["out"]
    if _trace:
        return full, res
    return full
